# revision 2
# baseline (speedup 1.0000x reference)
"""CRF forward on 8 Trainium2 cores — meet-in-the-middle bf16 chain.

answer_b = s_{len_b} = 1.T v_{len_b},  v_t = f_t * (A v_{t-1}).
Tags permuted so the dead END tag sits at partition SROW. A := exp(trans'-mu)
with row SROW := ones (sum row) and column SROW := e_SROW (self-loop weight 1).
Padded steps (f_t = e_SROW for t >= len) collapse v to s*e_SROW and hold s
exactly, so s_T is the answer for every lane. By bilinearity
s_T = r_HALF.T v_HALF with the backward functional r_{t-1} = Ab.T (f_t*r_t),
r_S = ones, Ab = A with column SROW := ones (sum monitor). Forward (512
steps) and backward (512 steps) chains run concurrently on each core ->
serial latency halves vs a single 1024-step scan. Sum-row renorms every 128
steps bound the fp range; the exact reciprocal factors are staged out and
the host adds the logs back. All tiles bf16 (PSUM fp32)."""
import sys
import numpy as np

sys.path.insert(0, "/opt/trn_rl_repo")

INF_MIN = -10000.0
B, S, T = 256, 1024, 128
START, END = T - 2, T - 1
SROW = 96
HALF = S // 2
NCORES = 8
BC = B // NCORES
RENORM = (128, 256, 384)
NREN = len(RENORM)
NST = 1 + 2 * NREN           # stage rows: dot + fwd recips + bwd recips
FCH = 2048                   # f chunk free size (64 steps x 32 lanes)
NCHK = (HALF * BC) // FCH

_cache = {}


def _build_program(half=HALF):
    RENORM_L = tuple(r for r in RENORM if r < half)
    import concourse.bass as bass
    import concourse.mybir as mybir
    from contextlib import ExitStack

    f32 = mybir.dt.float32
    bf16 = mybir.dt.bfloat16
    AF = mybir.ActivationFunctionType
    MUL = mybir.AluOpType.mult
    NW = 3
    NV = 4

    nc = bass.Bass()
    ewf_d = nc.declare_dram_parameter("ewf", [T, T], bf16, isOutput=False)
    ewb_d = nc.declare_dram_parameter("ewb", [T, T], bf16, isOutput=False)
    ff_d = nc.declare_dram_parameter("ff", [T, HALF * BC], bf16, isOutput=False)
    fb_d = nc.declare_dram_parameter("fb", [T, HALF * BC], bf16, isOutput=False)
    res_d = nc.declare_dram_parameter("res", [1, NST * BC], f32, isOutput=True)

    # --- emission-order bookkeeping (shared ground truth between engines) ---
    # dvef incs: DVE_f(t) for t=2..HALF, +1 scale after each fwd renorm t
    # dveb incs: DVE_b(k) for k=1..HALF, +1 scale after DVE_b(kappa+1), +1 qdot
    def dvef_after(t):            # incs once DVE_f(t) emitted (no scale yet)
        return (t - 1) + sum(1 for r in RENORM_L if r < t)

    def dvef_scaled(t):           # incs incl. scale at renorm t
        return (t - 1) + sum(1 for r in RENORM_L if r <= t)

    def dveb_after(k):            # incs once DVE_b(k) emitted
        return k + sum(1 for r in RENORM_L if r + 1 < k)

    def dveb_scaled(k):           # incs incl. scale of u_k (k = kappa+1)
        return k + sum(1 for r in RENORM_L if r + 1 <= k)

    es = ExitStack()
    with es:
        ewf = es.enter_context(nc.sbuf_tensor("ewf_sb", [T, T], bf16))
        ewb = es.enter_context(nc.sbuf_tensor("ewb_sb", [T, T], bf16))
        ffc = [es.enter_context(nc.sbuf_tensor(f"ff{c}", [T, FCH], bf16))
               for c in range(NCHK)]
        fbc = [es.enter_context(nc.sbuf_tensor(f"fb{c}", [T, FCH], bf16))
               for c in range(NCHK)]
        vf = [es.enter_context(nc.sbuf_tensor(f"vf{k}", [T, BC], bf16))
              for k in range(NV)]
        ub = [es.enter_context(nc.sbuf_tensor(f"ub{k}", [T, BC], bf16))
              for k in range(NV)]
        rone = es.enter_context(nc.sbuf_tensor("rone", [T, BC], bf16))
        ones_sb = es.enter_context(nc.sbuf_tensor("ones_sb", [1, T], f32))
        rcf = [es.enter_context(nc.sbuf_tensor(f"rcf{j}", [1, BC], f32))
               for j in range(NREN)]
        rcb = [es.enter_context(nc.sbuf_tensor(f"rcb{j}", [1, BC], f32))
               for j in range(NREN)]
        qdot = es.enter_context(nc.sbuf_tensor("qdot", [T, BC], bf16))
        stage = es.enter_context(nc.sbuf_tensor("stage", [1, NST * BC], f32))
        wf = [es.enter_context(nc.psum_tensor(f"wf{k}", [T, BC], f32))
              for k in range(NW)]
        rb = [es.enter_context(nc.psum_tensor(f"rb{k}", [T, BC], f32))
              for k in range(NW)]
        bc_f = es.enter_context(nc.psum_tensor("bc_f", [T, BC], f32))
        bc_b = es.enter_context(nc.psum_tensor("bc_b", [T, BC], f32))
        s_dma = es.enter_context(nc.semaphore("s_dma"))
        s_ini = es.enter_context(nc.semaphore("s_ini"))
        s_pef = es.enter_context(nc.semaphore("s_pef"))
        s_dvef = es.enter_context(nc.semaphore("s_dvef"))
        s_peb = es.enter_context(nc.semaphore("s_peb"))
        s_dveb = es.enter_context(nc.semaphore("s_dveb"))
        s_rc = es.enter_context(nc.semaphore("s_rc"))
        s_bc = es.enter_context(nc.semaphore("s_bc"))
        s_fin = es.enter_context(nc.semaphore("s_fin"))
        s_out = es.enter_context(nc.semaphore("s_out"))
        block = es.enter_context(nc.Block())

        # DMA order: ewf, ewb, then ff0, fb0, ff1, fb1, ...
        def ff_ready(c):
            return 16 * (3 + 2 * c)

        def fb_ready(c):
            return 16 * (4 + 2 * c)

        @block.sync
        def _(sync):
            # serialized issue: each wait pins completion order so the
            # counting-semaphore thresholds below are race-free
            n = 0
            sync.dma_start(ewf[:], ewf_d[:]).then_inc(s_dma, 16)
            n += 16
            sync.wait_ge(s_dma, n)
            sync.dma_start(ewb[:], ewb_d[:]).then_inc(s_dma, 16)
            n += 16
            sync.wait_ge(s_dma, n)
            for c in range(NCHK):
                sync.dma_start(ffc[c][:], ff_d[:, c * FCH:(c + 1) * FCH]
                               ).then_inc(s_dma, 16)
                n += 16
                sync.wait_ge(s_dma, n)
                sync.dma_start(fbc[c][:], fb_d[:, c * FCH:(c + 1) * FCH]
                               ).then_inc(s_dma, 16)
                n += 16
                sync.wait_ge(s_dma, n)
            sync.wait_ge(s_fin, 1)
            sync.dma_start(res_d[:], stage[:]).then_inc(s_out, 16)
            sync.wait_ge(s_out, 16)

        @block.vector
        def _(vector):
            vector.memset(ones_sb[:], 1.0)
            vector.memset(rone[:], 1.0).then_inc(s_ini, 1)
            rc_cnt = 0
            bc_cnt = 0
            for t in range(2, half + 1):
                k = t - 1
                # DVE_f(t): vf[t] = f_t * wf[t]
                col = (t - 1) * BC
                c0 = col // FCH
                vector.wait_ge(s_dma, ff_ready(c0))
                vector.wait_ge(s_pef, t - 1)
                vector.tensor_tensor(vf[t % NV][:],
                                     ffc[c0][:, col % FCH:col % FCH + BC],
                                     wf[t % NW][:], MUL).then_inc(s_dvef, 1)
                if t in RENORM_L:
                    j = RENORM_L.index(t)
                    # recip of sum row of wf[t] (= sum of v_{t-1})
                    vector.wait_ge(s_pef, t - 1)
                    vector.reciprocal(rcf[j][:], wf[t % NW][SROW:SROW + 1, :]
                                      ).then_inc(s_rc, 1)
                    rc_cnt += 1
                    # scale vf[t] by bcast (PE emits bc_f right after MM_f(t))
                    bc_cnt += 1
                    vector.wait_ge(s_bc, bc_cnt)
                    vector.wait_ge(s_dvef, dvef_after(t))
                    vector.tensor_tensor(vf[t % NV][:], vf[t % NV][:],
                                         bc_f[:], MUL).then_inc(s_dvef, 1)
                # DVE_b(k): u_k = f_bwd(k) * r  (r = rb[k-1] PSUM, or ones)
                colb = (k - 1) * BC
                c1_ = colb // FCH
                vector.wait_ge(s_dma, fb_ready(c1_))
                if k > 1:
                    vector.wait_ge(s_peb, k - 1)
                    rsrc = rb[(k - 1) % NW][:]
                else:
                    vector.wait_ge(s_ini, 1)
                    rsrc = rone[:]
                vector.tensor_tensor(ub[k % NV][:],
                                     fbc[c1_][:, colb % FCH:colb % FCH + BC],
                                     rsrc, MUL).then_inc(s_dveb, 1)
                if k in RENORM_L:
                    j = RENORM_L.index(k)
                    vector.wait_ge(s_peb, k)
                    vector.reciprocal(rcb[j][:], rb[k % NW][SROW:SROW + 1, :]
                                      ).then_inc(s_rc, 1)
                    rc_cnt += 1
                if (k - 1) in RENORM_L:
                    # scale u_k by bcast of 1/sum(r_{kappa}) with kappa = k-1
                    bc_cnt += 1
                    vector.wait_ge(s_bc, bc_cnt)
                    vector.wait_ge(s_dveb, dveb_after(k))
                    vector.tensor_tensor(ub[k % NV][:], ub[k % NV][:],
                                         bc_b[:], MUL).then_inc(s_dveb, 1)
            # tail: DVE_b(half), then qdot
            k = half
            colb = (k - 1) * BC
            c1_ = colb // FCH
            vector.wait_ge(s_dma, fb_ready(c1_))
            vector.wait_ge(s_peb, k - 1)
            vector.tensor_tensor(ub[k % NV][:],
                                 fbc[c1_][:, colb % FCH:colb % FCH + BC],
                                 rb[(k - 1) % NW][:], MUL).then_inc(s_dveb, 1)
            vector.wait_ge(s_peb, k)
            vector.wait_ge(s_dvef, dvef_scaled(half))
            vector.tensor_tensor(qdot[:], vf[half % NV][:],
                                 rb[half % NW][:], MUL).then_inc(s_dveb, 1)

        @block.tensor
        def _(pe):
            bc_cnt = 0
            rc_need = {}  # map: emission point -> rc count needed
            # rc incs in vector order: at t in RENORM: recip_f (after DVE_f);
            # at k=t-1 in RENORM: recip_b. Build the same running count:
            rc_at = {}
            rc = 0
            for t in range(2, half + 1):
                if t in RENORM_L:
                    rc += 1
                    rc_at[("f", t)] = rc
                if (t - 1) in RENORM_L:
                    rc += 1
                    rc_at[("b", t - 1)] = rc
            pe.wait_ge(s_dma, 16 * 2)
            pe.wait_ge(s_ini, 1)
            for t in range(2, half + 1):
                k = t - 1
                # MM_f(t)
                if t == 2:
                    pe.wait_ge(s_dma, ff_ready(0))
                    rhs = ffc[0][:, 0:BC]
                else:
                    pe.wait_ge(s_dvef, dvef_scaled(t - 1))
                    rhs = vf[(t - 1) % NV][:]
                pe.matmul(wf[t % NW][:], lhsT=ewf[:], rhs=rhs,
                          start=True, stop=True).then_inc(s_pef, 1)
                if t in RENORM_L:
                    # bc_f right after MM_f(t): scale_vf (vec idx t) needs it
                    j = RENORM_L.index(t)
                    pe.wait_ge(s_rc, rc_at[("f", t)])
                    pe.matmul(bc_f[:], lhsT=ones_sb[:], rhs=rcf[j][:],
                              start=True, stop=True).then_inc(s_bc, 1)
                    bc_cnt += 1
                if (t - 3) in RENORM_L:
                    # bc_b for bwd renorm at kappa=t-3: after MM_b(kappa)
                    # (PE idx kappa+2), before MM_b(kappa+1) (this idx)
                    j = RENORM_L.index(t - 3)
                    pe.wait_ge(s_rc, rc_at[("b", t - 3)])
                    pe.matmul(bc_b[:], lhsT=ones_sb[:], rhs=rcb[j][:],
                              start=True, stop=True).then_inc(s_bc, 1)
                    bc_cnt += 1
                # MM_b(k2 = t-2): one-index lag so it consumes u from the
                # PREVIOUS vector index (decouples the two chains' latencies)
                k2 = t - 2
                if k2 >= 1:
                    pe.wait_ge(s_dveb, dveb_scaled(k2))
                    pe.matmul(rb[k2 % NW][:], lhsT=ewb[:], rhs=ub[k2 % NV][:],
                              start=True, stop=True).then_inc(s_peb, 1)
            # tail: MM_b(half-1), MM_b(half), final sum MM
            for k2 in (half - 1, half):
                pe.wait_ge(s_dveb, dveb_scaled(k2))
                pe.matmul(rb[k2 % NW][:], lhsT=ewb[:], rhs=ub[k2 % NV][:],
                          start=True, stop=True).then_inc(s_peb, 1)
            pe.wait_ge(s_dveb, dveb_after(half) + 1)   # qdot inc
            pe.matmul(bc_f[:], lhsT=ewf[:], rhs=qdot[:],
                      start=True, stop=True).then_inc(s_bc, 1)

        @block.scalar
        def _(scalar):
            # off-critical-path: copy recips + final dot row to stage
            rc_at = {}
            rc = 0
            order = []
            for t in range(2, half + 1):
                if t in RENORM_L:
                    rc += 1
                    rc_at[("f", t)] = rc
                    order.append(("f", t))
                if (t - 1) in RENORM_L:
                    rc += 1
                    rc_at[("b", t - 1)] = rc
                    order.append(("b", t - 1))
            for typ, idx in order:
                j = RENORM_L.index(idx)
                scalar.wait_ge(s_rc, rc_at[(typ, idx)])
                if typ == "f":
                    scalar.activation(stage[:, (1 + j) * BC:(2 + j) * BC],
                                      rcf[j][:], AF.Copy)
                else:
                    scalar.activation(stage[:, (1 + NREN + j) * BC:
                                            (2 + NREN + j) * BC],
                                      rcb[j][:], AF.Copy)
            # final: all bcasts + 1 sum MM
            scalar.wait_ge(s_bc, 2 * len(RENORM_L) + 1)
            scalar.activation(stage[:, 0:BC], bc_f[SROW:SROW + 1, :],
                              AF.Copy).then_inc(s_fin, 1)
    return nc


def _host_constants(fp, tp):
    """g (step-1 fold), mu (mean log growth), c1 (scale) — float64 on 8 lanes."""
    alpha0 = np.full(T, INF_MIN)
    alpha0[START] = 0.0
    m0 = tp + alpha0[None, :]
    gmax = m0.max(axis=1, keepdims=True)
    g = gmax[:, 0] + np.log(np.exp(m0 - gmax).sum(axis=1))

    nb = 8
    A64 = np.exp(tp)
    a = fp[:nb, 0, :] + g[None, :]
    vv = np.exp(a - a.max(axis=1, keepdims=True)).T
    ac = a.max(axis=1)
    m_first = float((np.log(vv.sum(axis=0)) + ac).mean())
    for t in range(1, S):
        vv = np.exp(fp[:nb, t, :]).T * (A64 @ vv)
        m = vv.max(axis=0)
        vv /= m[None, :]
        ac += np.log(m)
    m_last = float((np.log(vv.sum(axis=0)) + ac).mean())
    mu = (m_last - m_first) / (S - 1)
    c1 = float(g.max())
    return g, mu, c1


def run(features, batch_len, transitions, trace=False):
    from concourse.bass_utils import run_bass_kernel_spmd
    import ml_dtypes

    features = np.asarray(features, dtype=np.float32)
    batch_len = np.asarray(batch_len, dtype=np.int32)
    transitions = np.asarray(transitions, dtype=np.float32)
    bft = ml_dtypes.bfloat16

    perm = np.arange(T)
    perm[SROW], perm[END] = END, SROW
    fp = features[:, :, perm].astype(np.float64)
    tp = transitions[perm][:, perm].astype(np.float64)
    g, mu, c1 = _host_constants(fp, tp)

    A = np.exp(tp - mu)
    A[SROW, :] = 1.0
    A[:, SROW] = 0.0
    A[SROW, SROW] = 1.0
    Ab = A.copy()
    Ab[:, SROW] = 1.0
    ewf = np.ascontiguousarray(A.T).astype(bft)    # lhsT fwd: out = A @ v
    ewb = np.ascontiguousarray(Ab).astype(bft)     # lhsT bwd: out = Ab.T @ u

    blen = batch_len.astype(np.int64)
    fexp = np.exp(fp).astype(np.float32)
    fexp[:, 0, :] = np.exp(fp[:, 0, :] + g[None, :] - c1)
    dead = np.arange(S)[None, :, None] >= blen[:, None, None]
    fexp = np.where(dead, 0.0, fexp)
    fexp[:, :, SROW] = np.where(dead[:, :, 0], 1.0, 0.0)
    fexp = fexp.astype(bft)

    in_maps = []
    for cid in range(NCORES):
        fc = fexp[cid * BC:(cid + 1) * BC]              # [32, 1024, 128]
        ffwd = fc[:, :HALF, :]                          # steps 1..512
        fbwd = fc[:, :HALF - 1:-1, :]                   # steps 1024..513
        ffwd = np.ascontiguousarray(ffwd.transpose(2, 1, 0)).reshape(T, HALF * BC)
        fbwd = np.ascontiguousarray(fbwd.transpose(2, 1, 0)).reshape(T, HALF * BC)
        in_maps.append({"ewf": ewf, "ewb": ewb, "ff": ffwd, "fb": fbwd})

    if "nc" not in _cache:
        _cache["nc"] = _build_program()
    res = run_bass_kernel_spmd(_cache["nc"], in_maps, list(range(NCORES)),
                               trace=trace)

    out = np.zeros(B, dtype=np.float32)
    for cid in range(NCORES):
        st = np.asarray(res.results[cid]["res"]).reshape(NST, BC
                                                         ).astype(np.float64)
        dot = st[0]
        corr = -np.log(st[1:]).sum(axis=0)   # staged values are reciprocals
        lb = blen[cid * BC:(cid + 1) * BC]
        out[cid * BC:(cid + 1) * BC] = (
            np.log(dot) + corr + c1 + (lb - 1) * mu - 10000.0
        ).astype(np.float32)
    return out, res


def kernel(features, batch_len, transitions):
    out, _ = run(features, batch_len, transitions, trace=False)
    return out


# revision 6
# speedup vs baseline: 1.0511x; 1.0511x over previous
"""CRF forward on 8 Trainium2 cores — meet-in-the-middle bf16 chain.

answer_b = s_{len_b} = 1.T v_{len_b},  v_t = f_t * (A v_{t-1}).
Tags permuted so the dead END tag sits at partition SROW. A := exp(trans'-mu)
with row SROW := ones (sum row) and column SROW := e_SROW (self-loop weight 1).
Padded steps (f_t = e_SROW for t >= len) collapse v to s*e_SROW and hold s
exactly, so s_T is the answer for every lane. By bilinearity
s_T = r_HALF.T v_HALF with the backward functional r_{t-1} = Ab.T (f_t*r_t),
r_S = ones, Ab = A with column SROW := ones (sum monitor). Forward (512
steps) and backward (512 steps) chains run concurrently on each core ->
serial latency halves vs a single 1024-step scan. Sum-row renorms every 128
steps bound the fp range; the exact reciprocal factors are staged out and
the host adds the logs back. All tiles bf16 (PSUM fp32)."""
import sys
import numpy as np

sys.path.insert(0, "/opt/trn_rl_repo")

INF_MIN = -10000.0
B, S, T = 256, 1024, 128
START, END = T - 2, T - 1
SROW = 96
HALF = S // 2
NCORES = 8
BC = B // NCORES
RENORM = (128, 256, 384)
NREN = len(RENORM)
NST = 1 + 2 * NREN           # stage rows: dot + fwd recips + bwd recips
FCH = 2048                   # f chunk free size (64 steps x 32 lanes)
NCHK = (HALF * BC) // FCH

_cache = {}


def _build_program(half=HALF):
    RENORM_L = tuple(r for r in RENORM if r < half)
    import concourse.bass as bass
    import concourse.mybir as mybir
    from contextlib import ExitStack

    f32 = mybir.dt.float32
    bf16 = mybir.dt.bfloat16
    AF = mybir.ActivationFunctionType
    MUL = mybir.AluOpType.mult
    NW = 3
    NV = 4

    nc = bass.Bass()
    ewf_d = nc.declare_dram_parameter("ewf", [T, T], bf16, isOutput=False)
    ewb_d = nc.declare_dram_parameter("ewb", [T, T], bf16, isOutput=False)
    ff_d = nc.declare_dram_parameter("ff", [T, HALF * BC], bf16, isOutput=False)
    fb_d = nc.declare_dram_parameter("fb", [T, HALF * BC], bf16, isOutput=False)
    res_d = nc.declare_dram_parameter("res", [1, NST * BC], f32, isOutput=True)

    # --- emission-order bookkeeping (shared ground truth between engines) ---
    # dvef incs: DVE_f(t) for t=2..HALF, +1 scale after each fwd renorm t
    # dveb incs: DVE_b(k) for k=1..HALF, +1 scale after DVE_b(kappa+1), +1 qdot
    def dvef_after(t):            # incs once DVE_f(t) emitted (no scale yet)
        return (t - 1) + sum(1 for r in RENORM_L if r < t)

    def dvef_scaled(t):           # incs incl. scale at renorm t
        return (t - 1) + sum(1 for r in RENORM_L if r <= t)

    def dveb_after(k):            # incs once DVE_b(k) emitted
        return k + sum(1 for r in RENORM_L if r + 1 < k)

    def dveb_scaled(k):           # incs incl. scale of u_k (k = kappa+1)
        return k + sum(1 for r in RENORM_L if r + 1 <= k)

    es = ExitStack()
    with es:
        ewf = es.enter_context(nc.sbuf_tensor("ewf_sb", [T, T], bf16))
        ewb = es.enter_context(nc.sbuf_tensor("ewb_sb", [T, T], bf16))
        ffc = [es.enter_context(nc.sbuf_tensor(f"ff{c}", [T, FCH], bf16))
               for c in range(NCHK)]
        fbc = [es.enter_context(nc.sbuf_tensor(f"fb{c}", [T, FCH], bf16))
               for c in range(NCHK)]
        vf = [es.enter_context(nc.sbuf_tensor(f"vf{k}", [T, BC], bf16))
              for k in range(NV)]
        ub = [es.enter_context(nc.sbuf_tensor(f"ub{k}", [T, BC], bf16))
              for k in range(NV)]
        rone = es.enter_context(nc.sbuf_tensor("rone", [T, BC], bf16))
        ones_sb = es.enter_context(nc.sbuf_tensor("ones_sb", [1, T], f32))
        rcf = [es.enter_context(nc.sbuf_tensor(f"rcf{j}", [1, BC], f32))
               for j in range(NREN)]
        rcb = [es.enter_context(nc.sbuf_tensor(f"rcb{j}", [1, BC], f32))
               for j in range(NREN)]
        qdot = es.enter_context(nc.sbuf_tensor("qdot", [T, BC], bf16))
        stage = es.enter_context(nc.sbuf_tensor("stage", [1, NST * BC], f32))
        wf = [es.enter_context(nc.psum_tensor(f"wf{k}", [T, BC], f32))
              for k in range(NW)]
        rb = [es.enter_context(nc.psum_tensor(f"rb{k}", [T, BC], f32))
              for k in range(NW)]
        bc_f = es.enter_context(nc.psum_tensor("bc_f", [T, BC], f32))
        bc_b = es.enter_context(nc.psum_tensor("bc_b", [T, BC], f32))
        s_dma = es.enter_context(nc.semaphore("s_dma"))
        s_ini = es.enter_context(nc.semaphore("s_ini"))
        s_pef = es.enter_context(nc.semaphore("s_pef"))
        s_dvef = es.enter_context(nc.semaphore("s_dvef"))
        s_peb = es.enter_context(nc.semaphore("s_peb"))
        s_dveb = es.enter_context(nc.semaphore("s_dveb"))
        s_rc = es.enter_context(nc.semaphore("s_rc"))
        s_bc = es.enter_context(nc.semaphore("s_bc"))
        s_fin = es.enter_context(nc.semaphore("s_fin"))
        s_out = es.enter_context(nc.semaphore("s_out"))
        block = es.enter_context(nc.Block())

        # DMA order: ewf, ewb, then ff0, fb0, ff1, fb1, ...
        def ff_ready(c):
            return 16 * (3 + 2 * c)

        def fb_ready(c):
            return 16 * (4 + 2 * c)

        @block.sync
        def _(sync):
            # serialized issue: each wait pins completion order so the
            # counting-semaphore thresholds below are race-free
            n = 0
            sync.dma_start(ewf[:], ewf_d[:]).then_inc(s_dma, 16)
            n += 16
            sync.wait_ge(s_dma, n)
            sync.dma_start(ewb[:], ewb_d[:]).then_inc(s_dma, 16)
            n += 16
            sync.wait_ge(s_dma, n)
            for c in range(NCHK):
                sync.dma_start(ffc[c][:], ff_d[:, c * FCH:(c + 1) * FCH]
                               ).then_inc(s_dma, 16)
                n += 16
                sync.wait_ge(s_dma, n)
                sync.dma_start(fbc[c][:], fb_d[:, c * FCH:(c + 1) * FCH]
                               ).then_inc(s_dma, 16)
                n += 16
                sync.wait_ge(s_dma, n)
            sync.wait_ge(s_fin, 1)
            sync.dma_start(res_d[:], stage[:]).then_inc(s_out, 16)
            sync.wait_ge(s_out, 16)

        @block.vector
        def _(vector):
            vector.memset(ones_sb[:], 1.0)
            vector.memset(rone[:], 1.0).then_inc(s_ini, 1)
            rc_cnt = 0
            bc_cnt = 0
            seen_ff = -1
            seen_fb = -1
            for t in range(2, half + 1):
                k = t - 1
                # DVE_f(t): vf[t] = f_t * wf[t]
                col = (t - 1) * BC
                c0 = col // FCH
                if c0 > seen_ff:
                    # chunk-entry wait only (sticky engine wait state);
                    # avoids a redundant wait instruction per step
                    vector.wait_ge(s_dma, ff_ready(c0))
                    seen_ff = c0
                vector.wait_ge(s_pef, t - 1)
                vector.tensor_tensor(vf[t % NV][:],
                                     ffc[c0][:, col % FCH:col % FCH + BC],
                                     wf[t % NW][:], MUL).then_inc(s_dvef, 1)
                if t in RENORM_L:
                    j = RENORM_L.index(t)
                    # recip of sum row of wf[t] (= sum of v_{t-1})
                    vector.wait_ge(s_pef, t - 1)
                    vector.reciprocal(rcf[j][:], wf[t % NW][SROW:SROW + 1, :]
                                      ).then_inc(s_rc, 1)
                    rc_cnt += 1
                    # scale vf[t] by bcast (PE emits bc_f right after MM_f(t))
                    bc_cnt += 1
                    vector.wait_ge(s_bc, bc_cnt)
                    vector.wait_ge(s_dvef, dvef_after(t))
                    vector.tensor_tensor(vf[t % NV][:], vf[t % NV][:],
                                         bc_f[:], MUL).then_inc(s_dvef, 1)
                # DVE_b(k): u_k = f_bwd(k) * r  (r = rb[k-1] PSUM, or ones)
                colb = (k - 1) * BC
                c1_ = colb // FCH
                if c1_ > seen_fb:
                    vector.wait_ge(s_dma, fb_ready(c1_))
                    seen_fb = c1_
                if k > 1:
                    vector.wait_ge(s_peb, k - 1)
                    rsrc = rb[(k - 1) % NW][:]
                else:
                    vector.wait_ge(s_ini, 1)
                    rsrc = rone[:]
                vector.tensor_tensor(ub[k % NV][:],
                                     fbc[c1_][:, colb % FCH:colb % FCH + BC],
                                     rsrc, MUL).then_inc(s_dveb, 1)
                if k in RENORM_L:
                    j = RENORM_L.index(k)
                    vector.wait_ge(s_peb, k)
                    vector.reciprocal(rcb[j][:], rb[k % NW][SROW:SROW + 1, :]
                                      ).then_inc(s_rc, 1)
                    rc_cnt += 1
                if (k - 1) in RENORM_L:
                    # scale u_k by bcast of 1/sum(r_{kappa}) with kappa = k-1
                    bc_cnt += 1
                    vector.wait_ge(s_bc, bc_cnt)
                    vector.wait_ge(s_dveb, dveb_after(k))
                    vector.tensor_tensor(ub[k % NV][:], ub[k % NV][:],
                                         bc_b[:], MUL).then_inc(s_dveb, 1)
            # tail: DVE_b(half), then qdot
            k = half
            colb = (k - 1) * BC
            c1_ = colb // FCH
            if colb // FCH > seen_fb:
                vector.wait_ge(s_dma, fb_ready(c1_))
            vector.wait_ge(s_peb, k - 1)
            vector.tensor_tensor(ub[k % NV][:],
                                 fbc[c1_][:, colb % FCH:colb % FCH + BC],
                                 rb[(k - 1) % NW][:], MUL).then_inc(s_dveb, 1)
            vector.wait_ge(s_peb, k)
            vector.wait_ge(s_dvef, dvef_scaled(half))
            vector.tensor_tensor(qdot[:], vf[half % NV][:],
                                 rb[half % NW][:], MUL).then_inc(s_dveb, 1)

        @block.tensor
        def _(pe):
            bc_cnt = 0
            rc_need = {}  # map: emission point -> rc count needed
            # rc incs in vector order: at t in RENORM: recip_f (after DVE_f);
            # at k=t-1 in RENORM: recip_b. Build the same running count:
            rc_at = {}
            rc = 0
            for t in range(2, half + 1):
                if t in RENORM_L:
                    rc += 1
                    rc_at[("f", t)] = rc
                if (t - 1) in RENORM_L:
                    rc += 1
                    rc_at[("b", t - 1)] = rc
            pe.wait_ge(s_dma, 16 * 2)
            pe.wait_ge(s_ini, 1)
            for t in range(2, half + 1):
                k = t - 1
                # MM_f(t)
                if t == 2:
                    pe.wait_ge(s_dma, ff_ready(0))
                    rhs = ffc[0][:, 0:BC]
                else:
                    pe.wait_ge(s_dvef, dvef_scaled(t - 1))
                    rhs = vf[(t - 1) % NV][:]
                pe.matmul(wf[t % NW][:], lhsT=ewf[:], rhs=rhs,
                          start=True, stop=True).then_inc(s_pef, 1)
                if t in RENORM_L:
                    # bc_f right after MM_f(t): scale_vf (vec idx t) needs it
                    j = RENORM_L.index(t)
                    pe.wait_ge(s_rc, rc_at[("f", t)])
                    pe.matmul(bc_f[:], lhsT=ones_sb[:], rhs=rcf[j][:],
                              start=True, stop=True).then_inc(s_bc, 1)
                    bc_cnt += 1
                if (t - 3) in RENORM_L:
                    # bc_b for bwd renorm at kappa=t-3: after MM_b(kappa)
                    # (PE idx kappa+2), before MM_b(kappa+1) (this idx)
                    j = RENORM_L.index(t - 3)
                    pe.wait_ge(s_rc, rc_at[("b", t - 3)])
                    pe.matmul(bc_b[:], lhsT=ones_sb[:], rhs=rcb[j][:],
                              start=True, stop=True).then_inc(s_bc, 1)
                    bc_cnt += 1
                # MM_b(k2 = t-2): one-index lag so it consumes u from the
                # PREVIOUS vector index (decouples the two chains' latencies)
                k2 = t - 2
                if k2 >= 1:
                    pe.wait_ge(s_dveb, dveb_scaled(k2))
                    pe.matmul(rb[k2 % NW][:], lhsT=ewb[:], rhs=ub[k2 % NV][:],
                              start=True, stop=True).then_inc(s_peb, 1)
            # tail: MM_b(half-1), MM_b(half), final sum MM
            for k2 in (half - 1, half):
                pe.wait_ge(s_dveb, dveb_scaled(k2))
                pe.matmul(rb[k2 % NW][:], lhsT=ewb[:], rhs=ub[k2 % NV][:],
                          start=True, stop=True).then_inc(s_peb, 1)
            pe.wait_ge(s_dveb, dveb_after(half) + 1)   # qdot inc
            pe.matmul(bc_f[:], lhsT=ewf[:], rhs=qdot[:],
                      start=True, stop=True).then_inc(s_bc, 1)

        @block.scalar
        def _(scalar):
            # off-critical-path: copy recips + final dot row to stage
            rc_at = {}
            rc = 0
            order = []
            for t in range(2, half + 1):
                if t in RENORM_L:
                    rc += 1
                    rc_at[("f", t)] = rc
                    order.append(("f", t))
                if (t - 1) in RENORM_L:
                    rc += 1
                    rc_at[("b", t - 1)] = rc
                    order.append(("b", t - 1))
            for typ, idx in order:
                j = RENORM_L.index(idx)
                scalar.wait_ge(s_rc, rc_at[(typ, idx)])
                if typ == "f":
                    scalar.activation(stage[:, (1 + j) * BC:(2 + j) * BC],
                                      rcf[j][:], AF.Copy)
                else:
                    scalar.activation(stage[:, (1 + NREN + j) * BC:
                                            (2 + NREN + j) * BC],
                                      rcb[j][:], AF.Copy)
            # final: all bcasts + 1 sum MM
            scalar.wait_ge(s_bc, 2 * len(RENORM_L) + 1)
            scalar.activation(stage[:, 0:BC], bc_f[SROW:SROW + 1, :],
                              AF.Copy).then_inc(s_fin, 1)
    return nc


def _host_constants(fp, tp):
    """g (step-1 fold), mu (mean log growth), c1 (scale) — float64 on 8 lanes."""
    alpha0 = np.full(T, INF_MIN)
    alpha0[START] = 0.0
    m0 = tp + alpha0[None, :]
    gmax = m0.max(axis=1, keepdims=True)
    g = gmax[:, 0] + np.log(np.exp(m0 - gmax).sum(axis=1))

    nb = 8
    A64 = np.exp(tp)
    a = fp[:nb, 0, :] + g[None, :]
    vv = np.exp(a - a.max(axis=1, keepdims=True)).T
    ac = a.max(axis=1)
    m_first = float((np.log(vv.sum(axis=0)) + ac).mean())
    for t in range(1, S):
        vv = np.exp(fp[:nb, t, :]).T * (A64 @ vv)
        m = vv.max(axis=0)
        vv /= m[None, :]
        ac += np.log(m)
    m_last = float((np.log(vv.sum(axis=0)) + ac).mean())
    mu = (m_last - m_first) / (S - 1)
    c1 = float(g.max())
    return g, mu, c1


def run(features, batch_len, transitions, trace=False):
    from concourse.bass_utils import run_bass_kernel_spmd
    import ml_dtypes

    features = np.asarray(features, dtype=np.float32)
    batch_len = np.asarray(batch_len, dtype=np.int32)
    transitions = np.asarray(transitions, dtype=np.float32)
    bft = ml_dtypes.bfloat16

    perm = np.arange(T)
    perm[SROW], perm[END] = END, SROW
    fp = features[:, :, perm].astype(np.float64)
    tp = transitions[perm][:, perm].astype(np.float64)
    g, mu, c1 = _host_constants(fp, tp)

    A = np.exp(tp - mu)
    A[SROW, :] = 1.0
    A[:, SROW] = 0.0
    A[SROW, SROW] = 1.0
    Ab = A.copy()
    Ab[:, SROW] = 1.0
    ewf = np.ascontiguousarray(A.T).astype(bft)    # lhsT fwd: out = A @ v
    ewb = np.ascontiguousarray(Ab).astype(bft)     # lhsT bwd: out = Ab.T @ u

    blen = batch_len.astype(np.int64)
    fexp = np.exp(fp).astype(np.float32)
    fexp[:, 0, :] = np.exp(fp[:, 0, :] + g[None, :] - c1)
    dead = np.arange(S)[None, :, None] >= blen[:, None, None]
    fexp = np.where(dead, 0.0, fexp)
    fexp[:, :, SROW] = np.where(dead[:, :, 0], 1.0, 0.0)
    fexp = fexp.astype(bft)

    in_maps = []
    for cid in range(NCORES):
        fc = fexp[cid * BC:(cid + 1) * BC]              # [32, 1024, 128]
        ffwd = fc[:, :HALF, :]                          # steps 1..512
        fbwd = fc[:, :HALF - 1:-1, :]                   # steps 1024..513
        ffwd = np.ascontiguousarray(ffwd.transpose(2, 1, 0)).reshape(T, HALF * BC)
        fbwd = np.ascontiguousarray(fbwd.transpose(2, 1, 0)).reshape(T, HALF * BC)
        in_maps.append({"ewf": ewf, "ewb": ewb, "ff": ffwd, "fb": fbwd})

    if "nc" not in _cache:
        _cache["nc"] = _build_program()
    res = None
    for attempt in range(3):
        try:
            res = run_bass_kernel_spmd(_cache["nc"], in_maps,
                                       list(range(NCORES)), trace=trace)
            break
        except Exception:
            # transient backend failures (device desync) — retry
            if attempt == 2:
                raise
            import time
            time.sleep(2.0)

    out = np.zeros(B, dtype=np.float32)
    for cid in range(NCORES):
        st = np.asarray(res.results[cid]["res"]).reshape(NST, BC
                                                         ).astype(np.float64)
        dot = st[0]
        corr = -np.log(st[1:]).sum(axis=0)   # staged values are reciprocals
        lb = blen[cid * BC:(cid + 1) * BC]
        out[cid * BC:(cid + 1) * BC] = (
            np.log(dot) + corr + c1 + (lb - 1) * mu - 10000.0
        ).astype(np.float32)
    return out, res


def kernel(features, batch_len, transitions):
    out, _ = run(features, batch_len, transitions, trace=False)
    return out
